# revision 1
# baseline (speedup 1.0000x reference)
"""Trainium2 Bass kernel for nn_Castle_34351148433552 (sparse_attention).

Sharding: 8 cores = 2 batches x 4 head-groups. Core c handles batch c//4,
heads 4*(c%4) .. 4*(c%4)+3. W_qkv is sliced column-wise per head group (with
the q-scale, and the silu-via-tanh 0.5 factor on vu, folded in on the host),
W_out row-wise. Each core computes its partial output projection transposed
([1024, 2048]); the host sums the 4 partials per batch and transposes back.

Device algorithm per (core, head), all in [d|c, n]-transposed layout:
  qkvT = (Wq_head^T x^T) via PE;  LT[j,k] = sigmoid(ku_j . qu_s_k) (j>k);
  T1T[j,i] = vu_j . qc_s_i (i>=j, pre-halved);  SuT'[k,i] = sum_j LT*T1T;
  silu(Su) = Su'*(1+tanh(Su')) (tanh shares the exp ACT table-set);
  scoresT = ScT - silu, causal fill -1e30;  expT;  AV via [vc|1] lhsT gives
  unnormalized out^T plus the softmax denominator in one PSUM accumulation;
  normalize; project through W_out rows.
"""

import os
import sys

import numpy as np

for _p in ("/opt/trn_rl_repo", os.path.expanduser("~/.axon_site/_ro/trn_rl_repo")):
    if os.path.isdir(_p) and _p not in sys.path:
        sys.path.insert(0, _p)
        break

H, D, NTOK, DIM = 16, 64, 2048, 1024
P = 128
NB = NTOK // P  # 16 row blocks
GW = 512        # i/k group width
NG = NTOK // GW  # 4 groups
HPC = 4         # heads per core
NCORES = 8
WHEAD = 6 * D   # 384 qkv columns per head


def _lt_offsets():
    off = {}
    o = 0
    for J in range(NB):
        for kg in range(J // 4 + 1):
            w = GW if kg < J // 4 else (J % 4 + 1) * P
            off[(J, kg)] = (o, w)
            o += w
    return off, o


_NC_CACHE = None


def build_nc():
    global _NC_CACHE
    if _NC_CACHE is not None:
        return _NC_CACHE

    import concourse.mybir as mybir
    import concourse.tile as tile
    from concourse import bacc
    from concourse.masks import make_identity

    dt = mybir.dt
    F32 = dt.float32
    F32R = dt.float32r
    AF = mybir.ActivationFunctionType
    ALU = mybir.AluOpType

    nc = bacc.Bacc(None, target_bir_lowering=False, debug=False)
    xT_d = nc.dram_tensor("xT", [DIM, NTOK], F32R, kind="ExternalInput")
    wq_d = nc.dram_tensor("wq", [DIM, HPC * WHEAD], F32R, kind="ExternalInput")
    wo_d = nc.dram_tensor("wo", [HPC * D, DIM], F32R, kind="ExternalInput")
    out_d = nc.dram_tensor("outT", [DIM, NTOK], F32, kind="ExternalOutput")

    lt_off, LTW = _lt_offsets()

    def r(ap):
        return ap

    phases = int(os.environ.get("KERNEL_PHASES", "6"))
    with tile.TileContext(nc) as tc:
        with (
            tc.tile_pool(name="const", bufs=1) as constp,
            tc.tile_pool(name="res", bufs=1) as resp,
            tc.tile_pool(name="xs", bufs=4) as xsp,
            tc.tile_pool(name="work", bufs=2) as workp,
            tc.tile_pool(name="outsb", bufs=2) as outsbp,
            tc.tile_pool(name="ps", bufs=7, space="PSUM") as psp,
            tc.tile_pool(name="pavp", bufs=1, space="PSUM") as pavp,
        ):
            # ---------- constants ----------
            ident = constp.tile([P, P], F32, tag="ident")
            make_identity(nc, ident)
            # trimask[p, F] = 1.0 iff F - p >= 384; slice at (3-o)*128 gives
            # the "keep f - p >= o*128" mask used for T1T diag blocks.
            trimask = constp.tile([P, 896], F32, tag="trimask")
            nc.gpsimd.memset(trimask, 1.0)
            nc.gpsimd.affine_select(
                out=trimask, in_=trimask, compare_op=ALU.is_ge, fill=0.0,
                base=-384, pattern=[[1, 896]], channel_multiplier=-1,
            )
            ones_col = constp.tile([1, D], F32, tag="ones")
            nc.gpsimd.memset(ones_col, 1.0)
            # strimask[p, F] = 1.0 iff p - F + 384 > 0; slice at 384 - o*128
            # gives "keep p - f + o*128 > 0" (strict j > k) for LT diag blocks
            strimask = constp.tile([P, 896], F32, tag="strimask")
            nc.gpsimd.memset(strimask, 1.0)
            nc.gpsimd.affine_select(
                out=strimask, in_=strimask, compare_op=ALU.is_gt, fill=0.0,
                base=384, pattern=[[-1, 896]], channel_multiplier=1,
            )
            # [vc | 1] stationary blocks, one [128, 65] slot per k-block.
            vc_ones = constp.tile([P, NB * (D + 1)], F32R, tag="vco")
            ones_stage = constp.tile([P, NB], F32, tag="onesstage")
            nc.gpsimd.memset(ones_stage, 1.0)
            nc.vector.tensor_copy(
                vc_ones.rearrange("p (k c) -> p k c", c=D + 1)[:, :, D],
                ones_stage)

            # ---------- resident tiles ----------
            # per-head transposed qkv (matmul lhsT/rhs base partitions must
            # match): t0=[qu|vu], t1=[ku|qc], t2=[vc|kc]
            qkvT = [resp.tile([P, NTOK], F32R, tag=f"qkvT{i}", name=f"qkvT{i}")
                    for i in range(3)]
            LT = resp.tile([P, LTW], F32R, tag="LT")
            T1T = resp.tile([P, NB * GW], F32R, tag="T1T")
            attn = [resp.tile([P, NTOK], F32R, tag=f"attn{i}", name=f"attn{i}")
                    for i in range(2)]
            wqres = resp.tile([P, 8 * WHEAD], F32R, tag="wqres")
            wores = [resp.tile([P, DIM], F32R, tag=f"wores{i}", name=f"wores{i}")
                     for i in range(2)]

            for i in range(2):
                nc.sync.dma_start(wores[i], wo_d[i * P:(i + 1) * P, :])

            for hh in range(HPC):
                # ---------- A: qkv projection for this head ----------
                for kc in range(8):
                    nc.sync.dma_start(
                        wqres[:, kc * WHEAD:(kc + 1) * WHEAD],
                        wq_d[kc * P:(kc + 1) * P, hh * WHEAD:(hh + 1) * WHEAD],
                    )
                for ng in range(NG):
                    ps = []
                    for ct in range(3):
                        ps.append(psp.tile([P, GW], F32, tag="ps", name=f"psq{ct}"))
                    for kc in range(8):
                        xt = xsp.tile([P, GW], F32R, tag="xt")
                        nc.sync.dma_start(
                            xt, xT_d[kc * P:(kc + 1) * P, ng * GW:(ng + 1) * GW])
                        for ct in range(3):
                            nc.tensor.matmul(
                                ps[ct],
                                r(wqres[:, kc * WHEAD + ct * P: kc * WHEAD + (ct + 1) * P]),
                                r(xt), start=(kc == 0), stop=(kc == 7),
                            )
                    for ct in range(3):
                        nc.scalar.copy(
                            qkvT[ct][:, ng * GW:(ng + 1) * GW], ps[ct])

                # ---------- C: LT = masked sigmoid(ku . qu_s) ----------
                # J-ascending so Su of early i-groups can start (subtile deps)
                for J in range(NB):
                    for kg in range(J // 4 + 1):
                        off, w = lt_off[(J, kg)]
                        pl = psp.tile([P, GW], F32, tag="ps")
                        nc.tensor.matmul(
                            pl[:, :w],
                            r(qkvT[1][0:D, J * P:(J + 1) * P]),
                            r(qkvT[0][0:D, kg * GW: kg * GW + w]),
                            start=True, stop=True,
                        )
                        nc.scalar.activation(
                            LT[:, off:off + w], pl[:, :w], AF.Sigmoid)
                        if kg == J // 4:
                            o2 = J % 4
                            nc.vector.tensor_tensor(
                                LT[:, off:off + w], LT[:, off:off + w],
                                strimask[:, 384 - o2 * P: 384 - o2 * P + w],
                                op=ALU.mult)

                # ---------- B: vc natural layout + ones columns ----------
                for kb in range(NB):
                    pt = psp.tile([P, D], F32, tag="ps")
                    nc.tensor.transpose(
                        pt, qkvT[2][0:D, kb * P:(kb + 1) * P].bitcast(F32),
                        ident[0:D, 0:D])
                    nc.vector.tensor_copy(
                        vc_ones[:, kb * (D + 1): kb * (D + 1) + D], pt)

                # ---------- D: attention, per 512-wide i-group ----------
                for ig in range(NG if phases >= 2 else 0):
                    nblk = 4 * ig + 4

                    # T1T strips (term1 transposed, pre-halved)
                    for J in range(nblk):
                        pt2 = psp.tile([P, GW], F32, tag="ps")
                        nc.tensor.matmul(
                            pt2,
                            r(qkvT[0][D:2 * D, J * P:(J + 1) * P]),
                            r(qkvT[1][D:2 * D, ig * GW:(ig + 1) * GW]),
                            start=True, stop=True,
                        )
                        dst = T1T[:, J * GW:(J + 1) * GW]
                        if J < 4 * ig:
                            nc.vector.tensor_copy(dst, pt2)
                        else:
                            o = J - 4 * ig
                            m = trimask[:, (3 - o) * P:(3 - o) * P + GW]
                            nc.vector.tensor_tensor(dst, pt2, m, op=ALU.mult)

                    if phases < 3:
                        continue
                    # merged scores pass per k-block: Su' accumulate, Sc,
                    # silu-via-tanh, subtract, causal fill, exp, AV accum
                    pav_t = pavp.tile([D + 1, GW], F32, tag="av")
                    for K in range(nblk):
                        psu = psp.tile([P, GW], F32, tag="ps")
                        for J in range(K, nblk):
                            o_, _w = lt_off[(J, K // 4)]
                            nc.tensor.matmul(
                                psu,
                                r(LT[:, o_ + (K % 4) * P: o_ + (K % 4 + 1) * P]),
                                r(T1T[:, J * GW:(J + 1) * GW]),
                                start=(J == K), stop=(J == nblk - 1),
                            )
                        psc = psp.tile([P, GW], F32, tag="ps")
                        nc.tensor.matmul(
                            psc,
                            r(qkvT[2][D:2 * D, K * P:(K + 1) * P]),
                            r(qkvT[1][D:2 * D, ig * GW:(ig + 1) * GW]),
                            start=True, stop=True,
                        )
                        if phases < 4:
                            continue
                        tnh = workp.tile([P, GW], F32, tag="tanh")
                        nc.scalar.activation(tnh, psu, AF.Tanh)
                        # silu(Su) in place: tnh = (tnh + 1) * Su'
                        nc.vector.scalar_tensor_tensor(
                            out=tnh, in0=tnh, scalar=1.0, in1=psu,
                            op0=ALU.add, op1=ALU.mult,
                        )
                        sct = workp.tile([P, GW], F32, tag="sct")
                        nc.vector.tensor_tensor(sct, psc, tnh, op=ALU.subtract)
                        if K >= 4 * ig:
                            nc.gpsimd.affine_select(
                                out=sct, in_=sct,
                                compare_op=ALU.is_ge, fill=-1e30,
                                base=ig * GW - K * P, pattern=[[1, GW]],
                                channel_multiplier=-1,
                            )
                        ext = workp.tile([P, GW], F32R, tag="ext")
                        nc.scalar.activation(ext, sct, AF.Exp)
                        nc.tensor.matmul(
                            pav_t,
                            r(vc_ones[:, K * (D + 1):(K + 1) * (D + 1)]),
                            r(ext),
                            start=(K == 0), stop=(K == nblk - 1),
                        )

                    if phases < 5:
                        continue
                    # stage AV out of PSUM immediately (frees the bank; the
                    # slow exact reciprocal then runs off the critical path)
                    avs = workp.tile([D + 1, GW], F32, tag="avs")
                    nc.vector.tensor_copy(avs, pav_t)
                    recip_t = workp.tile([1, GW], F32, tag="recip", bufs=1)
                    nc.vector.reciprocal(recip_t, avs[D:D + 1, :])
                    pbc = psp.tile([D, GW], F32, tag="ps")
                    nc.tensor.matmul(pbc, r(ones_col), r(recip_t), start=True, stop=True)
                    at = attn[hh // 2][(hh % 2) * D:(hh % 2 + 1) * D,
                                       ig * GW:(ig + 1) * GW]
                    nc.vector.tensor_tensor(at, avs[0:D, :], pbc, op=ALU.mult)

            # ---------- E: output projection (transposed) ----------
            # head pairs are stacked along partitions in attn[i]/wores[i],
            # so the c-contraction is two full-128-partition matmuls
            for dt_ in range(8 if phases >= 6 else 0):
                for ng in range(NG):
                    pp = psp.tile([P, GW], F32, tag="ps")
                    for i in range(2):
                        nc.tensor.matmul(
                            pp, r(wores[i][:, dt_ * P:(dt_ + 1) * P]),
                            r(attn[i][:, ng * GW:(ng + 1) * GW]),
                            start=(i == 0), stop=(i == 1))
                    ot = outsbp.tile([P, GW], F32, tag="ot")
                    if (dt_ * NG + ng) % 2 == 0:
                        nc.scalar.copy(ot, pp)
                    else:
                        nc.vector.tensor_copy(ot, pp)
                    nc.sync.dma_start(
                        out_d[dt_ * P:(dt_ + 1) * P, ng * GW:(ng + 1) * GW], ot)

    nc.compile()
    _NC_CACHE = nc
    return nc


def shard_inputs(x, W_qkv, W_out):
    """Host-side sharding: per-core input dicts."""
    x = np.asarray(x, np.float32)
    W_qkv = np.asarray(W_qkv, np.float32)
    W_out = np.asarray(W_out, np.float32)
    scale = D ** -0.5
    W6 = W_qkv.reshape(DIM, 6, H, D)
    in_maps = []
    xT = [np.ascontiguousarray(x[b].T) for b in range(2)]
    for c in range(NCORES):
        b, h0 = c // 4, 4 * (c % 4)
        Wc = W6[:, :, h0:h0 + HPC, :].transpose(0, 2, 1, 3).copy()  # [DIM,4,6,D]
        Wc[:, :, 0, :] *= scale  # qu
        Wc[:, :, 3, :] *= scale  # qc
        Wc[:, :, 2, :] *= 0.5    # vu -> Su' = Su/2 for silu-via-tanh
        # device c-order per head: [qu, vu, ku, qc, vc, kc]
        Wc = Wc[:, :, [0, 2, 1, 3, 5, 4], :]
        wo_c = np.ascontiguousarray(
            W_out.reshape(H, D, DIM)[h0:h0 + HPC].reshape(HPC * D, DIM))
        in_maps.append({
            "xT": xT[b],
            "wq": np.ascontiguousarray(Wc.reshape(DIM, HPC * WHEAD)),
            "wo": wo_c,
        })
    return in_maps


def unshard_output(results):
    """results: list of 8 dicts with 'outT' [1024, 2048] partials."""
    outs = []
    for b in range(2):
        acc = results[4 * b]["outT"].astype(np.float32).copy()
        for c in range(4 * b + 1, 4 * b + 4):
            acc += results[c]["outT"]
        outs.append(acc.T)
    return np.stack(outs).astype(np.float32)


def kernel(x, W_qkv, W_out):
    from concourse.bass_utils import run_bass_kernel_spmd

    in_maps = shard_inputs(x, W_qkv, W_out)
    nc = build_nc()
    res = run_bass_kernel_spmd(nc, in_maps, core_ids=list(range(NCORES)))
    return unshard_output(res.results)



# revision 29
# speedup vs baseline: 1.3979x; 1.3979x over previous
"""Trainium2 Bass kernel for nn_Castle_34351148433552 (sparse_attention).

Sharding: 8 cores = 2 batches x 4 head-groups (4 heads/core, as 2 pairs).
W_qkv sliced column-wise per head pair (scale folded on host), W_out
row-wise; each core emits a partial outT [1024, 2048]; host sums 4
partials per batch and transposes.

v2 design notes (vs v1 baseline @1.02ms):
  - All attention matmul operands in bf16 (FWL weight loads, half SBUF),
    PSUM accumulation stays fp32. Measured numpy quantization error 4e-3.
  - sigmoid(x) replaced by 0.5+0.5*tanh(x/2): the whole kernel then uses
    only the exp_and_others ACT table set (exp+tanh) -> zero table swaps
    and no ACT ordering constraints.
  - Head-pair row-tiling: the d=64-contraction T1T matmuls for the two
    heads of a pair run concurrently on PE row groups 0-1 / 2-3.
  - exp output (ext) overwrites the T1T slot it retires -> one shared
    [128, 16*512] bf16 buffer per head-slot.
  - softmax normalize moved off the PE critical path:
    reciprocal_approx_fast on the PSUM denominator row, one indicator
    matmul broadcasts both heads' recips, DVE multiplies from PSUM.
  - LT buffers (34KB bf16/head) double-buffered by head parity, with the
    LT build for head h+2 staggered to overlap the tail of head h's Su.
  - x staged once per (pair, ng) into an SBUF slab, reused by all 6
    qkv-column tiles.
"""

import os
import sys

import numpy as np

for _p in ("/opt/trn_rl_repo", os.path.expanduser("~/.axon_site/_ro/trn_rl_repo")):
    if os.path.isdir(_p) and _p not in sys.path:
        sys.path.insert(0, _p)
        break

H, D, NTOK, DIM = 16, 64, 2048, 1024
P = 128
NB = NTOK // P   # 16 row blocks
GW = 512         # i-group width
NG = NTOK // GW  # 4 groups
NCORES = 8
NPAIR = 2        # head pairs per core
WPAIR = 6 * P    # 768 qkv columns per pair


def _lt_offsets():
    off = {}
    o = 0
    for J in range(NB):
        for kg in range(J // 4 + 1):
            w = GW if kg < J // 4 else (J % 4 + 1) * P
            off[(J, kg)] = (o, w)
            o += w
    return off, o


_NC_CACHE = None


def build_nc():
    global _NC_CACHE
    if _NC_CACHE is not None:
        return _NC_CACHE

    import concourse.mybir as mybir
    import concourse.tile as tile
    from concourse import bacc
    from concourse.masks import make_identity

    dt = mybir.dt
    F32 = dt.float32
    F32R = dt.float32r
    BF16 = dt.bfloat16
    AF = mybir.ActivationFunctionType
    ALU = mybir.AluOpType

    nc = bacc.Bacc(None, target_bir_lowering=False, debug=False)
    xT_d = nc.dram_tensor("xT", [DIM, NTOK], BF16, kind="ExternalInput")
    wq_d = nc.dram_tensor("wq", [DIM, NPAIR * WPAIR], BF16, kind="ExternalInput")
    wo_d = nc.dram_tensor("wo", [NPAIR * P, DIM], BF16, kind="ExternalInput")
    out_d = nc.dram_tensor("outT", [DIM, NTOK], F32, kind="ExternalOutput")

    lt_off, LTW = _lt_offsets()

    with tile.TileContext(nc) as tc:
        with (
            tc.tile_pool(name="const", bufs=1) as constp,
            tc.tile_pool(name="res", bufs=1) as resp,
            tc.tile_pool(name="big", bufs=8) as bigp,
            tc.tile_pool(name="kc", bufs=2) as kcp,
            tc.tile_pool(name="vcf", bufs=1) as vcfp,
            tc.tile_pool(name="lt", bufs=2) as ltp,
            tc.tile_pool(name="t1", bufs=2) as t1p,
            tc.tile_pool(name="vco", bufs=1) as vcop,
            tc.tile_pool(name="wq", bufs=1) as wqp,
            tc.tile_pool(name="xsl", bufs=2) as xsp,

            tc.tile_pool(name="work", bufs=2) as workp,
            tc.tile_pool(name="outsb", bufs=2) as outp,
            tc.tile_pool(name="ps", bufs=5, space="PSUM") as psp,
            tc.tile_pool(name="pav", bufs=2, space="PSUM") as pavp,
            tc.tile_pool(name="pt", bufs=1, space="PSUM") as ptp,
        ):
            # ---------- constants ----------
            ident = constp.tile([P, P], F32, tag="ident")
            make_identity(nc, ident)


            wores = [resp.tile([P, DIM], BF16, tag=f"wo{i}", name=f"wores{i}")
                     for i in range(2)]
            for i in range(2):
                nc.sync.dma_start(wores[i], wo_d[i * P:(i + 1) * P, :])

            # per-pair resident handles
            qkvs = {}   # (pair, tt) -> [128, 2048] bf16, tt in QU,KU,VU,QC
            kcs = {}    # pair -> KC tile
            vcf = {}    # pair -> VC fp32 tile (for PE transpose)
            vcos = {}   # pair -> [vc|1|vc|1] stationary blocks
            t1s = {}    # head-parity slot -> T1T/ext tile (alloc per pair)
            lts = {}    # local head -> LT tile
            attns = {}  # pair -> attn tile

            QU, KU, VU, QC = 0, 1, 2, 3  # tile-type indices (KC=4, VC=5)

            def emit_A(p):
                wq = wqp.tile([P, 8 * WPAIR], BF16, tag="wq", name=f"wq{p}")
                for kc in range(8):
                    nc.sync.dma_start(
                        wq[:, kc * WPAIR:(kc + 1) * WPAIR],
                        wq_d[kc * P:(kc + 1) * P, p * WPAIR:(p + 1) * WPAIR])
                for tt in range(4):
                    qkvs[(p, tt)] = bigp.tile(
                        [P, NTOK], BF16, tag="big", name=f"qkv{p}_{tt}")
                kcs[p] = kcp.tile([P, NTOK], BF16, tag="kc", name=f"kc{p}")
                vcf[p] = vcfp.tile([P, NTOK], F32, tag="vcf", name=f"vcf{p}")
                for ng in range(NG):
                    xsl = xsp.tile([P, 8 * GW], BF16, tag="xsl", name=f"xsl{p}_{ng}")
                    for kc in range(8):
                        nc.sync.dma_start(
                            xsl[:, kc * GW:(kc + 1) * GW],
                            xT_d[kc * P:(kc + 1) * P, ng * GW:(ng + 1) * GW])
                    for tt in range(6):
                        ps = psp.tile([P, GW], F32, tag="ps", name=f"psq{tt}")
                        for kc in range(8):
                            nc.tensor.matmul(
                                ps,
                                wq[:, kc * WPAIR + tt * P: kc * WPAIR + (tt + 1) * P],
                                xsl[:, kc * GW:(kc + 1) * GW],
                                start=(kc == 0), stop=(kc == 7))
                        sl = slice(ng * GW, (ng + 1) * GW)
                        dst = (vcf[p][:, sl] if tt == 5 else
                               kcs[p][:, sl] if tt == 4 else
                               qkvs[(p, tt)][:, sl])
                        if (ng + tt) % 2:
                            nc.vector.tensor_copy(dst, ps)
                        else:
                            nc.scalar.copy(dst, ps)

            def emit_B(p):
                # per-K stationary blocks [vc_h0(64) | vc_h1(64) | ones(64)]:
                # the ones block makes the AV matmul broadcast the softmax
                # denominator onto PSUM partitions 64-127.
                if p == 0:
                    vco = vcop.tile([P, NB * 3 * D], BF16, tag="vco", name="vco")
                    nc.gpsimd.memset(
                        vco.rearrange("q (k g c) -> q k g c", g=3, c=D)[:, :, 1, :],
                        1.0)
                else:
                    vco = vcos[0]
                vcos[p] = vco
                vcov = vco.rearrange("q (k g c) -> q k g c", g=3, c=D)
                for kb in range(NB):
                    pt = ptp.tile([P, P], F32, tag="pt")
                    nc.tensor.transpose(
                        pt, vcf[p][:, kb * P:(kb + 1) * P], ident)
                    nc.scalar.copy(
                        vcov[:, kb, 0::2, :],
                        pt.rearrange("q (two c) -> q two c", two=2))

            def emit_C(h):
                """LT for local head h: 0.5 + 0.5*tanh(0.5 * qu_s.ku),
                strictly-lower (j>k) masked on diagonal blocks."""
                p, half = h // 2, h % 2
                hb = slice(64 * half, 64 * half + 64)
                lt = ltp.tile([P, LTW], BF16, tag="lt", name=f"lt{h}")
                lts[h] = lt
                for J in range(NB):
                    for kg in range(J // 4 + 1):
                        off, w = lt_off[(J, kg)]
                        ps = psp.tile([P, GW], F32, tag="ps", name="pslt")
                        nc.tensor.matmul(
                            ps[:, :w],
                            qkvs[(p, KU)][hb, J * P:(J + 1) * P],
                            qkvs[(p, QU)][hb, kg * GW: kg * GW + w],
                            start=True, stop=True)
                        sl = lt[:, off:off + w]
                        nc.scalar.activation(sl, ps[:, :w], AF.Tanh, scale=0.5)
                        nc.gpsimd.tensor_scalar(
                            out=sl, in0=sl, scalar1=1.0, scalar2=0.5,
                            op0=ALU.add, op1=ALU.mult)
                        if kg == J // 4:
                            nc.gpsimd.affine_select(
                                out=sl, in_=sl, compare_op=ALU.is_gt,
                                fill=0.0, base=(J % 4) * P,
                                pattern=[[-1, w]], channel_multiplier=1)

            def emit_D_ig(p, ig, before_h1=None):
                """One i-group of phase D for pair p."""
                nblk = 4 * ig + 4
                h0, h1 = 2 * p, 2 * p + 1
                # T1T strips, head-pair row-tiled
                for J in range(nblk):
                    pts = []
                    for half in range(2):
                        hb = slice(64 * half, 64 * half + 64)
                        pt2 = psp.tile([P, GW], F32, tag="ps", name=f"pst{half}")
                        nc.tensor.matmul(
                            pt2,
                            qkvs[(p, VU)][hb, J * P:(J + 1) * P],
                            qkvs[(p, QC)][hb, ig * GW:(ig + 1) * GW],
                            start=True, stop=True)
                        pts.append(pt2)
                    for half in range(2):
                        dst = t1s[half][:, J * GW:(J + 1) * GW]
                        nc.vector.tensor_copy(dst, pts[half])
                        if J >= 4 * ig:
                            # zero the j>i part of the diagonal strip in place
                            nc.gpsimd.affine_select(
                                out=dst, in_=dst, compare_op=ALU.is_ge,
                                fill=0.0, base=(4 * ig - J) * P,
                                pattern=[[1, GW]], channel_multiplier=-1)

                for half in range(2):
                    h = 2 * p + half
                    hb = slice(64 * half, 64 * half + 64)
                    t1 = t1s[half]
                    lt = lts[h]
                    pav = pavp.tile([P, GW], F32, tag="pav", name=f"pav{half}")
                    for K in range(nblk):
                        psu = psp.tile([P, GW], F32, tag="ps", name="psu")
                        for J in range(K, nblk):
                            o_, _w = lt_off[(J, K // 4)]
                            nc.tensor.matmul(
                                psu,
                                lt[:, o_ + (K % 4) * P: o_ + (K % 4 + 1) * P],
                                t1[:, J * GW:(J + 1) * GW],
                                start=(J == K), stop=(J == nblk - 1))
                        psc = psp.tile([P, GW], F32, tag="ps", name="psc")
                        nc.tensor.matmul(
                            psc,
                            kcs[p][hb, K * P:(K + 1) * P],
                            qkvs[(p, QC)][hb, ig * GW:(ig + 1) * GW],
                            start=True, stop=True)
                        tnh = workp.tile([P, GW], F32, tag="tnh")
                        nc.scalar.activation(tnh, psu, AF.Tanh)
                        # silu(Su) = Su' * (1 + tanh(Su')), Su' = Su/2 in psu
                        nc.vector.scalar_tensor_tensor(
                            out=tnh, in0=tnh, scalar=1.0, in1=psu,
                            op0=ALU.add, op1=ALU.mult)
                        sct = workp.tile([P, GW], F32, tag="sct")
                        nc.vector.tensor_tensor(sct, psc, tnh, op=ALU.subtract)
                        if K >= 4 * ig:
                            nc.gpsimd.affine_select(
                                out=sct, in_=sct, compare_op=ALU.is_ge,
                                fill=-1e30, base=ig * GW - K * P,
                                pattern=[[1, GW]], channel_multiplier=-1)
                        ext = t1[:, K * GW:(K + 1) * GW]
                        nc.scalar.activation(ext, sct, AF.Exp)
                        # half 0 lhsT = [vc|ones] -> attn on 0:64, denom 64:128
                        # half 1 lhsT = [ones|vc] -> denom on 0:64, attn 64:128
                        vslot = vcos[p][:, K * 3 * D + D * half:
                                        K * 3 * D + D * half + 2 * D]
                        nc.tensor.matmul(
                            pav, vslot, ext,
                            start=(K == 0), stop=(K == nblk - 1))
                    av_rows = pav[D:P, :] if half else pav[0:D, :]
                    dn_rows = pav[0:D, :] if half else pav[D:P, :]
                    # rafast misreads PSUM on HW -> stage denom to SBUF first
                    dstg = workp.tile([D, GW], F32, tag="dstg")
                    nc.vector.tensor_copy(dstg, dn_rows)
                    recB = workp.tile([D, GW], F32, tag="recB")
                    nc.vector.reciprocal_approx_fast(out=recB, in_=dstg)
                    nc.vector.tensor_tensor(
                        attns[p][64 * half:64 * half + 64, ig * GW:(ig + 1) * GW],
                        av_rows, recB, op=ALU.mult)
                    if half == 0 and before_h1 is not None:
                        before_h1()

            def emit_D(p, stagger=None):
                attns[p] = bigp.tile([P, NTOK], BF16, tag="big", name=f"attn{p}")
                for half in range(2):
                    t1s[half] = t1p.tile(
                        [P, NB * GW], BF16, tag="t1", name=f"t1_{p}_{half}")
                for ig in range(NG):
                    emit_D_ig(p, ig,
                              before_h1=stagger if ig == NG - 1 else None)

            def emit_E():
                for dt_ in range(8):
                    for ng in range(NG):
                        pp = psp.tile([P, GW], F32, tag="ps", name="pse")
                        for i in range(2):
                            nc.tensor.matmul(
                                pp, wores[i][:, dt_ * P:(dt_ + 1) * P],
                                attns[i][:, ng * GW:(ng + 1) * GW],
                                start=(i == 0), stop=(i == 1))
                        ot = outp.tile([P, GW], F32, tag="ot")
                        if (dt_ + ng) % 2:
                            nc.vector.tensor_copy(ot, pp)
                        else:
                            nc.scalar.copy(ot, pp)
                        nc.sync.dma_start(
                            out_d[dt_ * P:(dt_ + 1) * P, ng * GW:(ng + 1) * GW],
                            ot)

            # ---------- emission order ----------
            emit_A(0)
            emit_B(0)
            emit_C(0)
            emit_C(1)
            emit_A(1)   # dense PE filler for D(0)'s latency pockets
            emit_D(0, stagger=lambda: emit_C(2))
            emit_B(1)
            emit_C(3)
            emit_D(1)
            emit_E()

    nc.compile()
    _NC_CACHE = nc
    return nc


def shard_inputs(x, W_qkv, W_out):
    """Host-side sharding: per-core input dicts (bf16 operands)."""
    import ml_dtypes

    BF = ml_dtypes.bfloat16
    x = np.asarray(x, np.float32)
    W_qkv = np.asarray(W_qkv, np.float32)
    W_out = np.asarray(W_out, np.float32)
    scale = D ** -0.5
    W6 = W_qkv.reshape(DIM, 6, H, D).copy()
    W6[:, 0] *= scale  # qu
    W6[:, 3] *= scale  # qc
    W6[:, 2] *= 0.5    # vu -> Su' = Su/2 for silu-via-tanh
    xT = [np.ascontiguousarray(x[b].T).astype(BF) for b in range(2)]
    in_maps = []
    for c in range(NCORES):
        b, h0 = c // 4, 4 * (c % 4)
        wq = np.empty((DIM, NPAIR * WPAIR), np.float32)
        for p in range(NPAIR):
            ha, hb = h0 + 2 * p, h0 + 2 * p + 1
            for tt in range(6):
                col = p * WPAIR + tt * P
                wq[:, col:col + D] = W6[:, tt, ha, :]
                wq[:, col + D:col + 2 * D] = W6[:, tt, hb, :]
        wo = np.ascontiguousarray(
            W_out.reshape(H, D, DIM)[h0:h0 + 4].reshape(NPAIR * P, DIM))
        in_maps.append({
            "xT": xT[b],
            "wq": wq.astype(BF),
            "wo": wo.astype(BF),
        })
    return in_maps


def unshard_output(results):
    """results: list of 8 dicts with 'outT' [1024, 2048] partials."""
    outs = []
    for b in range(2):
        acc = results[4 * b]["outT"].astype(np.float32).copy()
        for c in range(4 * b + 1, 4 * b + 4):
            acc += results[c]["outT"]
        outs.append(acc.T)
    return np.stack(outs).astype(np.float32)


def kernel(x, W_qkv, W_out):
    from concourse.bass_utils import run_bass_kernel_spmd

    in_maps = shard_inputs(x, W_qkv, W_out)
    nc = build_nc()
    res = run_bass_kernel_spmd(nc, in_maps, core_ids=list(range(NCORES)))
    return unshard_output(res.results)


# revision 34
# speedup vs baseline: 1.4432x; 1.0324x over previous
"""Trainium2 Bass kernel for nn_Castle_34351148433552 (sparse_attention).

Sharding: 8 cores = 2 batches x 4 head-groups (4 heads/core, as 2 pairs).
W_qkv sliced column-wise per head pair (scale folded on host), W_out
row-wise; each core emits a partial outT [1024, 2048]; host sums 4
partials per batch and transposes.

v2 design notes (vs v1 baseline @1.02ms):
  - All attention matmul operands in bf16 (FWL weight loads, half SBUF),
    PSUM accumulation stays fp32. Measured numpy quantization error 4e-3.
  - sigmoid(x) replaced by 0.5+0.5*tanh(x/2): the whole kernel then uses
    only the exp_and_others ACT table set (exp+tanh) -> zero table swaps
    and no ACT ordering constraints.
  - Head-pair row-tiling: the d=64-contraction T1T matmuls for the two
    heads of a pair run concurrently on PE row groups 0-1 / 2-3.
  - exp output (ext) overwrites the T1T slot it retires -> one shared
    [128, 16*512] bf16 buffer per head-slot.
  - softmax normalize moved off the PE critical path:
    reciprocal_approx_fast on the PSUM denominator row, one indicator
    matmul broadcasts both heads' recips, DVE multiplies from PSUM.
  - LT buffers (34KB bf16/head) double-buffered by head parity, with the
    LT build for head h+2 staggered to overlap the tail of head h's Su.
  - x staged once per (pair, ng) into an SBUF slab, reused by all 6
    qkv-column tiles.
"""

import os
import sys

import numpy as np

for _p in ("/opt/trn_rl_repo", os.path.expanduser("~/.axon_site/_ro/trn_rl_repo")):
    if os.path.isdir(_p) and _p not in sys.path:
        sys.path.insert(0, _p)
        break

H, D, NTOK, DIM = 16, 64, 2048, 1024
P = 128
NB = NTOK // P   # 16 row blocks
GW = 512         # i-group width
NG = NTOK // GW  # 4 groups
NCORES = 8
NPAIR = 2        # head pairs per core
WPAIR = 6 * P    # 768 qkv columns per pair


def _lt_offsets():
    off = {}
    o = 0
    for J in range(NB):
        for kg in range(J // 4 + 1):
            w = GW if kg < J // 4 else (J % 4 + 1) * P
            off[(J, kg)] = (o, w)
            o += w
    return off, o


_NC_CACHE = None


def build_nc():
    global _NC_CACHE
    if _NC_CACHE is not None:
        return _NC_CACHE

    import concourse.mybir as mybir
    import concourse.tile as tile
    from concourse import bacc
    from concourse.masks import make_identity

    dt = mybir.dt
    F32 = dt.float32
    F32R = dt.float32r
    BF16 = dt.bfloat16
    AF = mybir.ActivationFunctionType
    ALU = mybir.AluOpType

    nc = bacc.Bacc(None, target_bir_lowering=False, debug=False)
    xT_d = nc.dram_tensor("xT", [DIM, NTOK], BF16, kind="ExternalInput")
    wq_d = nc.dram_tensor("wq", [DIM, NPAIR * WPAIR], BF16, kind="ExternalInput")
    wo_d = nc.dram_tensor("wo", [NPAIR * P, DIM], BF16, kind="ExternalInput")
    out_d = nc.dram_tensor("outT", [DIM, NTOK], F32, kind="ExternalOutput")

    lt_off, LTW = _lt_offsets()

    with tile.TileContext(nc) as tc:
        with (
            tc.tile_pool(name="const", bufs=1) as constp,
            tc.tile_pool(name="res", bufs=1) as resp,
            tc.tile_pool(name="big", bufs=8) as bigp,
            tc.tile_pool(name="kc", bufs=2) as kcp,
            tc.tile_pool(name="vcf", bufs=1) as vcfp,
            tc.tile_pool(name="lt", bufs=2) as ltp,
            tc.tile_pool(name="t1", bufs=2) as t1p,
            tc.tile_pool(name="vco", bufs=1) as vcop,
            tc.tile_pool(name="wq", bufs=1) as wqp,
            tc.tile_pool(name="xsl", bufs=2) as xsp,

            tc.tile_pool(name="work", bufs=2) as workp,
            tc.tile_pool(name="outsb", bufs=2) as outp,
            tc.tile_pool(name="ps", bufs=4, space="PSUM") as psp,
            tc.tile_pool(name="pav", bufs=2, space="PSUM") as pavp,
            # A-phase accumulators get their own pool: in the shared pool
            # their allocations would queue (FIFO rotation) behind every
            # pending-tanh C tile, so A could never fill C's PE gaps.
            tc.tile_pool(name="psA", bufs=2, space="PSUM") as psap,
        ):
            # ---------- constants ----------
            ident = constp.tile([P, P], F32, tag="ident")
            make_identity(nc, ident)


            wores = [resp.tile([P, DIM], BF16, tag=f"wo{i}", name=f"wores{i}")
                     for i in range(2)]
            for i in range(2):
                nc.sync.dma_start(wores[i], wo_d[i * P:(i + 1) * P, :])

            # per-pair resident handles
            qkvs = {}   # (pair, tt) -> [128, 2048] bf16, tt in QU,KU,VU,QC
            kcs = {}    # pair -> KC tile
            vcf = {}    # pair -> VC fp32 tile (for PE transpose)
            vcos = {}   # pair -> [vc|1|vc|1] stationary blocks
            t1s = {}    # head-parity slot -> T1T/ext tile (alloc per pair)
            lts = {}    # local head -> LT tile
            attns = {}  # pair -> attn tile

            QU, KU, VU, QC = 0, 1, 2, 3  # tile-type indices (KC=4, VC=5)

            def emit_A(p):
                wq = wqp.tile([P, 8 * WPAIR], BF16, tag="wq", name=f"wq{p}")
                for kc in range(8):
                    nc.sync.dma_start(
                        wq[:, kc * WPAIR:(kc + 1) * WPAIR],
                        wq_d[kc * P:(kc + 1) * P, p * WPAIR:(p + 1) * WPAIR])
                for tt in range(4):
                    qkvs[(p, tt)] = bigp.tile(
                        [P, NTOK], BF16, tag="big", name=f"qkv{p}_{tt}")
                kcs[p] = kcp.tile([P, NTOK], BF16, tag="kc", name=f"kc{p}")
                vcf[p] = vcfp.tile([P, NTOK], F32, tag="vcf", name=f"vcf{p}")
                for ng in range(NG):
                    xsl = xsp.tile([P, 8 * GW], BF16, tag="xsl", name=f"xsl{p}_{ng}")
                    for kc in range(8):
                        nc.sync.dma_start(
                            xsl[:, kc * GW:(kc + 1) * GW],
                            xT_d[kc * P:(kc + 1) * P, ng * GW:(ng + 1) * GW])
                    for tt in range(6):
                        ps = psap.tile([P, GW], F32, tag="psq", name=f"psq{tt}")
                        for kc in range(8):
                            nc.tensor.matmul(
                                ps,
                                wq[:, kc * WPAIR + tt * P: kc * WPAIR + (tt + 1) * P],
                                xsl[:, kc * GW:(kc + 1) * GW],
                                start=(kc == 0), stop=(kc == 7))
                        sl = slice(ng * GW, (ng + 1) * GW)
                        dst = (vcf[p][:, sl] if tt == 5 else
                               kcs[p][:, sl] if tt == 4 else
                               qkvs[(p, tt)][:, sl])
                        if (ng + tt) % 2:
                            nc.vector.tensor_copy(dst, ps)
                        else:
                            nc.scalar.copy(dst, ps)

            def emit_B(p):
                # per-K stationary blocks [vc_h0(64) | vc_h1(64) | ones(64)]:
                # the ones block makes the AV matmul broadcast the softmax
                # denominator onto PSUM partitions 64-127.
                if p == 0:
                    vco = vcop.tile([P, NB * 3 * D], BF16, tag="vco", name="vco")
                    nc.gpsimd.memset(
                        vco.rearrange("q (k g c) -> q k g c", g=3, c=D)[:, :, 1, :],
                        1.0)
                else:
                    vco = vcos[0]
                vcos[p] = vco
                vcov = vco.rearrange("q (k g c) -> q k g c", g=3, c=D)
                for kb in range(NB):
                    pt = psap.tile([P, GW], F32, tag="psq", name="pt")[:, 0:P]
                    nc.tensor.transpose(
                        pt, vcf[p][:, kb * P:(kb + 1) * P], ident)
                    nc.scalar.copy(
                        vcov[:, kb, 0::2, :],
                        pt.rearrange("q (two c) -> q two c", two=2))

            def emit_C(h):
                """LT for local head h: 0.5 + 0.5*tanh(0.5 * qu_s.ku),
                strictly-lower (j>k) masked on diagonal blocks."""
                p, half = h // 2, h % 2
                hb = slice(64 * half, 64 * half + 64)
                lt = ltp.tile([P, LTW], BF16, tag="lt", name=f"lt{h}")
                lts[h] = lt
                for J in range(NB):
                    for kg in range(J // 4 + 1):
                        off, w = lt_off[(J, kg)]
                        ps = psp.tile([P, GW], F32, tag="ps", name="pslt")
                        nc.tensor.matmul(
                            ps[:, :w],
                            qkvs[(p, KU)][hb, J * P:(J + 1) * P],
                            qkvs[(p, QU)][hb, kg * GW: kg * GW + w],
                            start=True, stop=True)
                        sl = lt[:, off:off + w]
                        nc.scalar.activation(sl, ps[:, :w], AF.Tanh, scale=0.5)
                        nc.gpsimd.tensor_scalar(
                            out=sl, in0=sl, scalar1=1.0, scalar2=0.5,
                            op0=ALU.add, op1=ALU.mult)
                        if kg == J // 4:
                            nc.gpsimd.affine_select(
                                out=sl, in_=sl, compare_op=ALU.is_gt,
                                fill=0.0, base=(J % 4) * P,
                                pattern=[[-1, w]], channel_multiplier=1)

            def emit_D_ig(p, ig, before_h1=None):
                """One i-group of phase D for pair p."""
                nblk = 4 * ig + 4
                h0, h1 = 2 * p, 2 * p + 1
                # T1T strips, head-pair row-tiled
                for J in range(nblk):
                    pts = []
                    for half in range(2):
                        hb = slice(64 * half, 64 * half + 64)
                        pt2 = psp.tile([P, GW], F32, tag="ps", name=f"pst{half}")
                        nc.tensor.matmul(
                            pt2,
                            qkvs[(p, VU)][hb, J * P:(J + 1) * P],
                            qkvs[(p, QC)][hb, ig * GW:(ig + 1) * GW],
                            start=True, stop=True)
                        pts.append(pt2)
                    for half in range(2):
                        dst = t1s[half][:, J * GW:(J + 1) * GW]
                        nc.vector.tensor_copy(dst, pts[half])
                        if J >= 4 * ig:
                            # zero the j>i part of the diagonal strip in place
                            nc.gpsimd.affine_select(
                                out=dst, in_=dst, compare_op=ALU.is_ge,
                                fill=0.0, base=(4 * ig - J) * P,
                                pattern=[[1, GW]], channel_multiplier=-1)

                for half in range(2):
                    h = 2 * p + half
                    hb = slice(64 * half, 64 * half + 64)
                    t1 = t1s[half]
                    lt = lts[h]
                    pav = pavp.tile([P, GW], F32, tag="pav", name=f"pav{half}")
                    for K in range(nblk):
                        psu = psp.tile([P, GW], F32, tag="ps", name="psu")
                        for J in range(K, nblk):
                            o_, _w = lt_off[(J, K // 4)]
                            nc.tensor.matmul(
                                psu,
                                lt[:, o_ + (K % 4) * P: o_ + (K % 4 + 1) * P],
                                t1[:, J * GW:(J + 1) * GW],
                                start=(J == K), stop=(J == nblk - 1))
                        psc = psp.tile([P, GW], F32, tag="ps", name="psc")
                        nc.tensor.matmul(
                            psc,
                            kcs[p][hb, K * P:(K + 1) * P],
                            qkvs[(p, QC)][hb, ig * GW:(ig + 1) * GW],
                            start=True, stop=True)
                        tnh = workp.tile([P, GW], F32, tag="tnh")
                        nc.scalar.activation(tnh, psu, AF.Tanh)
                        # silu(Su) = Su' * (1 + tanh(Su')), Su' = Su/2 in psu
                        nc.vector.scalar_tensor_tensor(
                            out=tnh, in0=tnh, scalar=1.0, in1=psu,
                            op0=ALU.add, op1=ALU.mult)
                        sct = workp.tile([P, GW], F32, tag="sct")
                        nc.vector.tensor_tensor(sct, psc, tnh, op=ALU.subtract)
                        if K >= 4 * ig:
                            nc.gpsimd.affine_select(
                                out=sct, in_=sct, compare_op=ALU.is_ge,
                                fill=-1e30, base=ig * GW - K * P,
                                pattern=[[1, GW]], channel_multiplier=-1)
                        ext = t1[:, K * GW:(K + 1) * GW]
                        nc.scalar.activation(ext, sct, AF.Exp)
                        # half 0 lhsT = [vc|ones] -> attn on 0:64, denom 64:128
                        # half 1 lhsT = [ones|vc] -> denom on 0:64, attn 64:128
                        vslot = vcos[p][:, K * 3 * D + D * half:
                                        K * 3 * D + D * half + 2 * D]
                        nc.tensor.matmul(
                            pav, vslot, ext,
                            start=(K == 0), stop=(K == nblk - 1))
                    av_rows = pav[D:P, :] if half else pav[0:D, :]
                    dn_rows = pav[0:D, :] if half else pav[D:P, :]
                    # rafast misreads PSUM on HW -> stage denom to SBUF first
                    dstg = workp.tile([D, GW], F32, tag="dstg")
                    nc.vector.tensor_copy(dstg, dn_rows)
                    recB = workp.tile([D, GW], F32, tag="recB")
                    nc.vector.reciprocal_approx_fast(out=recB, in_=dstg)
                    nc.vector.tensor_tensor(
                        attns[p][64 * half:64 * half + 64, ig * GW:(ig + 1) * GW],
                        av_rows, recB, op=ALU.mult)
                    if half == 0 and before_h1 is not None:
                        before_h1()

            def emit_D(p, stagger=None, after_ig=None):
                attns[p] = bigp.tile([P, NTOK], BF16, tag="big", name=f"attn{p}")
                for half in range(2):
                    t1s[half] = t1p.tile(
                        [P, NB * GW], BF16, tag="t1", name=f"t1_{p}_{half}")
                for ig in range(NG):
                    emit_D_ig(p, ig,
                              before_h1=stagger if ig == NG - 1 else None)
                    if after_ig is not None:
                        after_ig(ig)

            def emit_E(ng):
                """Output projection for one 512-wide token group."""
                for dt_ in range(8):
                    pp = psap.tile([P, GW], F32, tag="psq", name="pse")
                    for i in range(2):
                        nc.tensor.matmul(
                            pp, wores[i][:, dt_ * P:(dt_ + 1) * P],
                            attns[i][:, ng * GW:(ng + 1) * GW],
                            start=(i == 0), stop=(i == 1))
                    ot = outp.tile([P, GW], F32, tag="ot")
                    if (dt_ + ng) % 2:
                        nc.vector.tensor_copy(ot, pp)
                    else:
                        nc.scalar.copy(ot, pp)
                    nc.sync.dma_start(
                        out_d[dt_ * P:(dt_ + 1) * P, ng * GW:(ng + 1) * GW],
                        ot)

            # ---------- emission order ----------
            emit_A(0)
            emit_B(0)
            emit_C(0)
            emit_C(1)
            emit_A(1)   # dense PE filler for D(0)'s latency pockets
            emit_D(0, stagger=lambda: emit_C(2))
            emit_B(1)
            emit_C(3)
            emit_D(1, after_ig=emit_E)

    nc.compile()
    _NC_CACHE = nc
    return nc


def shard_inputs(x, W_qkv, W_out):
    """Host-side sharding: per-core input dicts (bf16 operands)."""
    import ml_dtypes

    BF = ml_dtypes.bfloat16
    x = np.asarray(x, np.float32)
    W_qkv = np.asarray(W_qkv, np.float32)
    W_out = np.asarray(W_out, np.float32)
    scale = D ** -0.5
    W6 = W_qkv.reshape(DIM, 6, H, D).copy()
    W6[:, 0] *= scale  # qu
    W6[:, 3] *= scale  # qc
    W6[:, 2] *= 0.5    # vu -> Su' = Su/2 for silu-via-tanh
    xT = [np.ascontiguousarray(x[b].T).astype(BF) for b in range(2)]
    in_maps = []
    for c in range(NCORES):
        b, h0 = c // 4, 4 * (c % 4)
        wq = np.empty((DIM, NPAIR * WPAIR), np.float32)
        for p in range(NPAIR):
            ha, hb = h0 + 2 * p, h0 + 2 * p + 1
            for tt in range(6):
                col = p * WPAIR + tt * P
                wq[:, col:col + D] = W6[:, tt, ha, :]
                wq[:, col + D:col + 2 * D] = W6[:, tt, hb, :]
        wo = np.ascontiguousarray(
            W_out.reshape(H, D, DIM)[h0:h0 + 4].reshape(NPAIR * P, DIM))
        in_maps.append({
            "xT": xT[b],
            "wq": wq.astype(BF),
            "wo": wo.astype(BF),
        })
    return in_maps


def unshard_output(results):
    """results: list of 8 dicts with 'outT' [1024, 2048] partials."""
    outs = []
    for b in range(2):
        acc = results[4 * b]["outT"].astype(np.float32).copy()
        for c in range(4 * b + 1, 4 * b + 4):
            acc += results[c]["outT"]
        outs.append(acc.T)
    return np.stack(outs).astype(np.float32)


def kernel(x, W_qkv, W_out):
    from concourse.bass_utils import run_bass_kernel_spmd

    in_maps = shard_inputs(x, W_qkv, W_out)
    nc = build_nc()
    res = run_bass_kernel_spmd(nc, in_maps, core_ids=list(range(NCORES)))
    return unshard_output(res.results)
